# revision 22
# baseline (speedup 1.0000x reference)
"""EntityAwareAttention Trainium2 kernel.

Single-head attention (B=4, S=4096, H=768) with a per-key-column additive
entity bias and key mask:

    q = x @ Wq.T + bq ; k = x @ Wk.T + bk ; v = x @ Wv.T + bv
    scores = q @ k.T / sqrt(H) + col_add[None, :]      (col_add per key column)
    ctx = softmax(scores) @ v

Sharding: 8 cores = 4 batches x 2 query-halves.  The key axis is ROTATED
per core (host-side) so each core's own query columns sit at [0, QH) —
one compiled program serves both halves, and the G projection reads its
queries straight out of the global X.T tile (no duplicate xtq stream).
All matmuls bf16 with fp32 PSUM accumulation.

Device tricks (everything PE-bound, ~bf16 tensor-engine roofline):
  * Fused QK: scores = X @ M @ X.T with M = Wq.T@Wk/sqrt(H) precomputed on
    the host, G = X_q @ M on device (queries only).  The K projection
    disappears; the scores stationary operand is raw X.T.  bq/bk cross
    terms are either constant per query row (softmax-invariant, dropped) or
    a per-key term X@d (d = Wk.T@bq/sqrt(H)) folded into the exp bias
    (emitted only when bq != 0).
  * Fused PV: ctx = (P @ X) @ Wv.T.  The V projection over 4096 keys
    becomes a post-projection over this core's 2048 queries (half cost);
    P is contracted against raw X in natural layout.
  * Scores are computed TRANSPOSED (S.T[k, q], k on partitions): the
    per-key bias/mask is a per-partition activation bias fused into Exp,
    and P.T = exp(S.T) feeds the P@X matmul directly as the moving operand
    -> zero on-chip transposes.
  * max-subtraction is skipped: scores here are O(1)-bounded, exp cannot
    overflow fp32, softmax is shift-invariant.
  * Softmax normalizer: l = column-sum of P.T via vector-engine partial
    sums; gpsimd all-reduces across partitions; 1/l = Exp(-Ln(l)) on the
    scalar engine; applied during PSUM->SBUF evacuation.
  * Head pipelining: the G-projection query chunks are interleaved with
    the first score tiles of query-chunk 0 (which ride the same arriving
    X.T chunks), so the PE stays busy through the initial HBM stream and
    the HAM clock never re-throttles.  ~8us of junk warmup matmuls bridge
    the initial DMA latency at the cold 1.2GHz clock (sized so stragglers
    whose HBM stream lags never idle past one 3.4us HAM window).
  * Inputs move as a handful of fat multi-MB 3D-AP DMA transfers (per-call
    HWDGE overhead otherwise throttles the head); output is staged and
    DMA'd as fp16 (host upcasts) and the last query chunk is processed in
    two halves, so the post-compute output drain is minimal.
"""
import math

import numpy as np
import ml_dtypes

import concourse.bass as bass
import concourse.bacc as bacc
import concourse.tile as tile
from concourse import mybir
from concourse.bass import ts
from concourse import bass_isa
from concourse.bass_utils import run_bass_kernel_spmd

P = 128
F32 = mybir.dt.float32
F16 = mybir.dt.float16
BF16 = mybir.dt.bfloat16
AF = mybir.ActivationFunctionType

WARM_MMS = 16


def build_attention_bass(S, H, QH, QC=512, bv_nonzero=False, bq_nonzero=False):
    HT = H // P           # h/o tiles
    KT = S // P           # key tiles
    NQC = QH // QC        # query chunks
    assert QC == 512 and QH * 2 == S

    nc = bacc.Bacc(trn_type="TRN2")

    xt_d = nc.dram_tensor("xt", [P, HT, S], BF16, kind="ExternalInput")
    xn_d = nc.dram_tensor("xn", [P, KT, H], BF16, kind="ExternalInput")
    m_d = nc.dram_tensor("m", [HT, P, H], BF16, kind="ExternalInput")
    wvt_d = nc.dram_tensor("wvt", [P, HT, H], BF16, kind="ExternalInput")
    col_d = nc.dram_tensor("col", [P, KT], F32, kind="ExternalInput")
    if bq_nonzero:
        dvec_d = nc.dram_tensor("dvec", [P, HT], BF16, kind="ExternalInput")
    if bv_nonzero:
        bv_d = nc.dram_tensor("bv2", [P, HT], F32, kind="ExternalInput")
    out_d = nc.dram_tensor("out", [HT, P, QH], F16, kind="ExternalOutput")

    with tile.TileContext(nc) as tc:
        with (
            tc.tile_pool(name="persist", bufs=1) as persist,
            tc.tile_pool(name="small", bufs=1) as small,
            tc.tile_pool(name="ptp", bufs=1) as ptp,
            tc.tile_pool(name="stp", bufs=2, space="PSUM") as stp,
            tc.tile_pool(name="ctxp", bufs=3, space="PSUM") as ctxp,
            tc.tile_pool(name="prjp", bufs=3, space="PSUM") as prjp,
            tc.tile_pool(name="osb", bufs=4) as osb,
            tc.tile_pool(name="usb", bufs=2) as usb,
            tc.tile_pool(name="lsb", bufs=1) as lsb,
        ):
            xt_sb = persist.tile([P, HT, S], BF16, tag="xt")   # raw X.T, global
            xn_sb = persist.tile([P, KT, H], BF16, tag="xn")   # raw X, natural
            gt_sb = persist.tile([P, HT, QH], BF16, tag="gt")  # G.T = (X@M).T
            wv_sb = persist.tile([P, HT, H], BF16, tag="wv")   # Wv.T
            m_sb = persist.tile([P, HT, H], BF16, tag="m")     # M

            colb = small.tile([P, KT], F32, tag="colb")
            nc.sync.dma_start(colb, col_d[:, :])
            if bv_nonzero:
                bv_sb = small.tile([P, HT], F32, tag="bv_sb")
                nc.sync.dma_start(bv_sb, bv_d[:, :])
            if bq_nonzero:
                d_sb = small.tile([P, HT], BF16, tag="d_sb")
                nc.sync.dma_start(d_sb, dvec_d[:, :])

            # ---- DMA stream, ordered by first consumption ----
            # Few, fat transfers: per-call HWDGE overhead throttled the head
            # when this was ~120 small calls.  m is host-transposed to
            # [ot, P, ht*128+c] so one 1536B-line transfer delivers the full
            # stationary set of G output-block ot; G group (qc=0, ot=0)
            # needs only m[:, 0, :] + xt chunk 0, and subsequent ot groups
            # pipeline on the m[ot] stream.
            nc.sync.dma_start(m_sb[:, 0, :], m_d[0, :, :])
            # chunk 0 in two calls so G(qc0, ot0, half-a) gates on ~590KB
            nc.sync.dma_start(xt_sb[:, :, 0 : QC // 2], xt_d[:, :, 0 : QC // 2])
            nc.sync.dma_start(xt_sb[:, :, QC // 2 : QC], xt_d[:, :, QC // 2 : QC])
            for ot in range(1, HT):
                nc.sync.dma_start(m_sb[:, ot, :], m_d[ot, :, :])
            # own-half X.T chunks 1..3 (feed G qc and scores kt 4qc..4qc+3)
            for xc in range(1, NQC):
                nc.sync.dma_start(xt_sb[:, :, ts(xc, QC)], xt_d[:, :, ts(xc, QC)])
            # other-half X.T in two chunks (scores qc0 kt 16..31)
            nc.sync.dma_start(
                xt_sb[:, :, QH : QH + QH // 2], xt_d[:, :, QH : QH + QH // 2]
            )
            nc.sync.dma_start(xt_sb[:, :, QH + QH // 2 : S], xt_d[:, :, QH + QH // 2 : S])
            # raw X (natural) for P@X, in two column halves so PV groups
            # ht 0..2 only gate on the first half
            nc.sync.dma_start(xn_sb[:, :, 0 : 3 * P], xn_d[:, :, 0 : 3 * P])
            nc.sync.dma_start(xn_sb[:, :, 3 * P : H], xn_d[:, :, 3 * P : H])
            # Wv last (first used by out-proj qc0)
            nc.sync.dma_start(wv_sb[:, :, :], wvt_d[:, :, :])

            # ~6us of dummy matmuls while the first DMAs land: the PE clock
            # is HAM-throttled to 1.2GHz until it has been busy for one
            # ~3.4us activity window, so warm it up on junk data and the
            # real work starts at 2.4GHz.
            warm = small.tile([P, QC], BF16, tag="warm")
            nc.vector.memset(warm, 0.0)
            wps = stp.tile([P, QC], F32, tag="st", name="wps")
            for i in range(WARM_MMS):
                nc.tensor.matmul(
                    wps, warm[:, 0:P], warm, start=(i == 0), stop=(i == WARM_MMS - 1)
                )

            pts = [None] * NQC

            def score_group(qc, kt):
                st_ps = stp.tile([P, QC], F32, tag="st", name="st_ps")
                for ot in range(HT):
                    nc.tensor.matmul(
                        st_ps,
                        xt_sb[:, ot, ts(kt, P)],
                        gt_sb[:, ot, ts(qc, QC)],
                        start=(ot == 0),
                        stop=(ot == HT - 1),
                    )
                nc.scalar.activation(
                    pts[qc][:, kt, :], st_ps, AF.Exp,
                    bias=colb[:, kt : kt + 1], scale=1.0,
                )

            # ---- head: G projection interleaved with scores(qc=0) of the
            # own-half key tiles that ride the same arriving X.T chunks ----
            pts[0] = ptp.tile([P, KT, QC], BF16, tag="pt", name="pt")

            def g_group(qc, ot, lo, w):
                pps = stp.tile([P, QC], F32, tag="st", name="pps")
                for ht in range(HT):
                    nc.tensor.matmul(
                        pps[:, 0:w],
                        m_sb[:, ot, ts(ht, P)],
                        xt_sb[:, ht, qc * QC + lo : qc * QC + lo + w],
                        start=(ht == 0),
                        stop=(ht == HT - 1),
                    )
                nc.vector.tensor_copy(
                    gt_sb[:, ot, qc * QC + lo : qc * QC + lo + w], pps[:, 0:w]
                )

            for qc in range(NQC):
                for ot in range(HT):
                    if qc == 0:
                        # halve the first chunk so the very first group only
                        # gates on ~590KB of the arriving HBM stream
                        g_group(qc, ot, 0, QC // 2)
                        g_group(qc, ot, QC // 2, QC // 2)
                    else:
                        g_group(qc, ot, 0, QC)
                for kt in range(4 * qc, 4 * qc + 4):
                    score_group(0, kt)

            if bq_nonzero:
                # per-key scalar c[k] = X[k] . d folded into the exp bias
                for kt in range(KT):
                    cpps = prjp.tile([P, 1], F32, tag="prj", name="cpps")
                    for ht in range(HT):
                        nc.tensor.matmul(
                            cpps,
                            xt_sb[:, ht, ts(kt, P)],
                            d_sb[:, ht : ht + 1],
                            start=(ht == 0),
                            stop=(ht == HT - 1),
                        )
                    nc.vector.tensor_tensor(
                        colb[:, kt : kt + 1], colb[:, kt : kt + 1], cpps,
                        mybir.AluOpType.add,
                    )

            # ---- attention ----
            for qc in range(NQC):
                if qc > 0:
                    pts[qc] = ptp.tile([P, KT, QC], BF16, tag="pt", name="pt")
                    for kt in range(KT):
                        score_group(qc, kt)
                else:
                    for kt in range(16, KT):
                        score_group(0, kt)
                pt = pts[qc]

                # l[q] = sum_k P.T[k, q]: partial sums on the (idle)
                # vector engine
                lacc = lsb.tile([P, QC], F32, tag="lacc", name="lacc", bufs=2)
                nc.vector.tensor_copy(lacc, pt[:, 0, :])
                for kt in range(1, KT):
                    nc.vector.tensor_tensor(
                        lacc, lacc, pt[:, kt, :], mybir.AluOpType.add
                    )

                # softmax normalizer, entirely off the PE: gpsimd
                # all-reduces lacc across partitions (result in every
                # partition), scalar does 1/l = Exp(-Ln(l)) elementwise.
                lbc = lsb.tile([P, QC], F32, tag="lbc", name="lbc")
                nc.gpsimd.partition_all_reduce(
                    lbc, lacc, 128, bass_isa.ReduceOp.add
                )
                lnl = lsb.tile([P, QC], F32, tag="lnl", name="lnl")
                nc.scalar.activation(lnl, lbc, AF.Ln, scale=1.0)
                bc_sb = lsb.tile([P, QC], F32, tag="bc_sb", name="bc_sb", bufs=2)
                nc.scalar.activation(bc_sb, lnl, AF.Exp, scale=-1.0)

                # U.T[h, q] = X.T-natural @ P.T (P contracted against raw
                # X; Wv applied afterwards to 2048 queries, not 4096 keys)
                # ctx.T[o, q] = Wv @ U.T; normalize + bv on evacuation.
                # The final chunk is processed in two query halves so the
                # last output DMA left dangling after the last matmul is
                # ~65KB, not ~786KB.
                def pv_proj(lo, w):
                    u_sb = usb.tile([P, HT, QC], BF16, tag="u", name="u_sb")
                    for ht in range(HT):
                        ups = ctxp.tile([P, QC], F32, tag="u_ps", name="ups")
                        for kt in range(KT):
                            nc.tensor.matmul(
                                ups[:, 0:w],
                                xn_sb[:, kt, ts(ht, P)],
                                pt[:, kt, lo : lo + w],
                                start=(kt == 0),
                                stop=(kt == KT - 1),
                            )
                        nc.any.tensor_copy(u_sb[:, ht, 0:w], ups[:, 0:w])
                    for ot in range(HT):
                        cps = prjp.tile([P, QC], F32, tag="prj", name="cps")
                        for ht in range(HT):
                            nc.tensor.matmul(
                                cps[:, 0:w],
                                wv_sb[:, ht, ts(ot, P)],
                                u_sb[:, ht, 0:w],
                                start=(ht == 0),
                                stop=(ht == HT - 1),
                            )
                        o_sb = osb.tile([P, QC], F16, tag="o", name="o_sb")
                        nc.vector.tensor_tensor(
                            o_sb[:, 0:w], cps[:, 0:w], bc_sb[:, lo : lo + w],
                            mybir.AluOpType.mult,
                        )
                        if bv_nonzero:
                            nc.vector.tensor_scalar_add(
                                o_sb[:, 0:w], o_sb[:, 0:w], bv_sb[:, ot : ot + 1]
                            )
                        nc.sync.dma_start(
                            out_d[ot, :, qc * QC + lo : qc * QC + lo + w],
                            o_sb[:, 0:w],
                        )

                if qc == NQC - 1:
                    pv_proj(0, QC // 2)
                    pv_proj(QC // 2, QC // 2)
                else:
                    pv_proj(0, QC)
    nc.finalize()
    return nc


# ------------------------- host side -------------------------

_NC_CACHE = {}
TRACE = False
TRACE_CORES = [0]
_LAST_RESULTS = None


def _get_nc(S, H, QH, bv_nonzero, bq_nonzero):
    key = (S, H, QH, bv_nonzero, bq_nonzero)
    if key not in _NC_CACHE:
        _NC_CACHE[key] = build_attention_bass(
            S, H, QH, bv_nonzero=bv_nonzero, bq_nonzero=bq_nonzero
        )
    return _NC_CACHE[key]


def kernel(hidden_states, attention_mask, entity_positions, Wq, bq, Wk, bk, Wv, bv):
    hs = np.asarray(hidden_states, dtype=np.float32)
    am = np.asarray(attention_mask, dtype=np.float32)
    ep = np.asarray(entity_positions)
    Wq = np.asarray(Wq, dtype=np.float32)
    Wk = np.asarray(Wk, dtype=np.float32)
    Wv = np.asarray(Wv, dtype=np.float32)
    bq = np.asarray(bq, dtype=np.float32)
    bv = np.asarray(bv, dtype=np.float32)
    # bk only shifts each query row's scores by a constant -> softmax-invariant

    B, S, H = hs.shape
    QH = S // 2
    HT = H // P
    KT = S // P
    OKT = QH // P
    scale = 1.0 / math.sqrt(H)

    # per-key-column additive term: entity bias (+1 per entity occurrence,
    # duplicates accumulate) + mask
    bias_cols = np.zeros((B, S), dtype=np.float32)
    np.add.at(bias_cols, (np.arange(B)[:, None], ep.astype(np.int64)), 1.0)
    col_add = bias_cols + (1.0 - am) * (-10000.0)

    M = (Wq.T @ Wk) * scale                      # [h, h']
    bv_nonzero = bool(np.any(bv != 0.0))
    bq_nonzero = bool(np.any(bq != 0.0))

    # m transposed to [ot, p, ht*128+c] = M[ht*128+p, ot*128+c]: one fat
    # contiguous transfer per G output block
    m_t = np.ascontiguousarray(
        M.reshape(HT, P, HT, P).transpose(2, 1, 0, 3).reshape(HT, P, H)
    )
    shared = {
        "m": m_t.astype(ml_dtypes.bfloat16),
        "wvt": np.ascontiguousarray(
            Wv.T.reshape(HT, P, H).transpose(1, 0, 2)
        ).astype(ml_dtypes.bfloat16),
    }
    if bq_nonzero:
        dvec = (Wk.T @ bq) * scale               # [h]
        shared["dvec"] = np.ascontiguousarray(
            dvec.reshape(HT, P).T.astype(ml_dtypes.bfloat16)
        )
    if bv_nonzero:
        shared["bv2"] = np.ascontiguousarray(bv.reshape(HT, P).T, dtype=np.float32)

    n_cores = 2 * B
    xt_fulls = [
        np.ascontiguousarray(
            hs[b].T.reshape(HT, P, S).transpose(1, 0, 2)
        ).astype(ml_dtypes.bfloat16)
        for b in range(B)
    ]
    xn_fulls = [
        np.ascontiguousarray(
            hs[b].reshape(KT, P, H).transpose(1, 0, 2)
        ).astype(ml_dtypes.bfloat16)
        for b in range(B)
    ]
    col_ts = [
        np.ascontiguousarray(col_add[b].reshape(KT, P).T, dtype=np.float32)
        for b in range(B)
    ]
    in_maps = []
    for core in range(n_cores):
        b, half = core // 2, core % 2
        if half == 0:
            d = {"xt": xt_fulls[b], "xn": xn_fulls[b], "col": col_ts[b]}
        else:
            # rotate the key axis so this core's queries sit at [0, QH)
            d = {
                "xt": np.ascontiguousarray(
                    np.concatenate(
                        [xt_fulls[b][:, :, QH:], xt_fulls[b][:, :, :QH]], axis=2
                    )
                ),
                "xn": np.ascontiguousarray(
                    np.concatenate([xn_fulls[b][:, OKT:], xn_fulls[b][:, :OKT]], axis=1)
                ),
                "col": np.ascontiguousarray(
                    np.concatenate([col_ts[b][:, OKT:], col_ts[b][:, :OKT]], axis=1)
                ),
            }
        d.update(shared)
        in_maps.append(d)

    nc = _get_nc(S, H, QH, bv_nonzero, bq_nonzero)
    kw = {}
    if TRACE:
        kw = dict(trace=True, trace_cores=list(TRACE_CORES))
    # the accelerator occasionally throws a transient
    # NRT_EXEC_UNIT_UNRECOVERABLE; a clean retry succeeds
    last_exc = None
    for _attempt in range(3):
        try:
            res = run_bass_kernel_spmd(
                nc, in_maps, core_ids=list(range(n_cores)), **kw
            )
            break
        except Exception as e:  # noqa: BLE001
            last_exc = e
    else:
        raise last_exc
    global _LAST_RESULTS
    _LAST_RESULTS = res

    out = np.empty((B, S, H), dtype=np.float32)
    for core in range(n_cores):
        b, half = core // 2, core % 2
        ctx_t = res.results[core]["out"].astype(np.float32).reshape(H, QH)  # [o, q]
        out[b, half * QH : (half + 1) * QH, :] = ctx_t.T
    return out


# revision 23
# speedup vs baseline: 1.0867x; 1.0867x over previous
"""EntityAwareAttention Trainium2 kernel.

Single-head attention (B=4, S=4096, H=768) with a per-key-column additive
entity bias and key mask:

    q = x @ Wq.T + bq ; k = x @ Wk.T + bk ; v = x @ Wv.T + bv
    scores = q @ k.T / sqrt(H) + col_add[None, :]      (col_add per key column)
    ctx = softmax(scores) @ v

Sharding: 8 cores = 4 batches x 2 query-halves.  The key axis is ROTATED
per core (host-side) so each core's own query columns sit at [0, QH) —
one compiled program serves both halves, and the G projection reads its
queries straight out of the global X.T tile (no duplicate xtq stream).
All matmuls bf16 with fp32 PSUM accumulation.

Device tricks (everything PE-bound, ~bf16 tensor-engine roofline):
  * Fused QK: scores = X @ M @ X.T with M = Wq.T@Wk/sqrt(H) precomputed on
    the host, G = X_q @ M on device (queries only).  The K projection
    disappears; the scores stationary operand is raw X.T.  bq/bk cross
    terms are either constant per query row (softmax-invariant, dropped) or
    a per-key term X@d (d = Wk.T@bq/sqrt(H)) folded into the exp bias
    (emitted only when bq != 0).
  * Fused PV: ctx = (P @ X) @ Wv.T.  The V projection over 4096 keys
    becomes a post-projection over this core's 2048 queries (half cost);
    P is contracted against raw X in natural layout.
  * Scores are computed TRANSPOSED (S.T[k, q], k on partitions): the
    per-key bias/mask is a per-partition activation bias fused into Exp,
    and P.T = exp(S.T) feeds the P@X matmul directly as the moving operand
    -> zero on-chip transposes.
  * max-subtraction is skipped: scores here are O(1)-bounded, exp cannot
    overflow fp32, softmax is shift-invariant.
  * Softmax normalizer: l = column-sum of P.T via vector-engine partial
    sums; gpsimd all-reduces across partitions; 1/l = Exp(-Ln(l)) on the
    scalar engine; applied during PSUM->SBUF evacuation.
  * Head pipelining: the G-projection query chunks are interleaved with
    the first score tiles of query-chunk 0 (which ride the same arriving
    X.T chunks), so the PE stays busy through the initial HBM stream and
    the HAM clock never re-throttles.  ~8us of junk warmup matmuls bridge
    the initial DMA latency at the cold 1.2GHz clock (sized so stragglers
    whose HBM stream lags never idle past one 3.4us HAM window).
  * Inputs move as a handful of fat multi-MB 3D-AP DMA transfers (per-call
    HWDGE overhead otherwise throttles the head); output is staged and
    DMA'd as fp16 (host upcasts) and the last query chunk is processed in
    two halves, so the post-compute output drain is minimal.
"""
import math

import numpy as np
import ml_dtypes

import concourse.bass as bass
import concourse.bacc as bacc
import concourse.tile as tile
from concourse import mybir
from concourse.bass import ts
from concourse import bass_isa
from concourse.bass_utils import run_bass_kernel_spmd

P = 128
F32 = mybir.dt.float32
F16 = mybir.dt.float16
BF16 = mybir.dt.bfloat16
AF = mybir.ActivationFunctionType

WARM_MMS = 20


def build_attention_bass(S, H, QH, QC=512, bv_nonzero=False, bq_nonzero=False):
    HT = H // P           # h/o tiles
    KT = S // P           # key tiles
    NQC = QH // QC        # query chunks
    assert QC == 512 and QH * 2 == S

    nc = bacc.Bacc(trn_type="TRN2")

    xt_d = nc.dram_tensor("xt", [P, HT, S], BF16, kind="ExternalInput")
    xn_d = nc.dram_tensor("xn", [P, KT, H], BF16, kind="ExternalInput")
    m_d = nc.dram_tensor("m", [HT, P, H], BF16, kind="ExternalInput")
    wvt_d = nc.dram_tensor("wvt", [P, HT, H], BF16, kind="ExternalInput")
    col_d = nc.dram_tensor("col", [P, KT], F32, kind="ExternalInput")
    if bq_nonzero:
        dvec_d = nc.dram_tensor("dvec", [P, HT], BF16, kind="ExternalInput")
    if bv_nonzero:
        bv_d = nc.dram_tensor("bv2", [P, HT], F32, kind="ExternalInput")
    out_d = nc.dram_tensor("out", [HT, P, QH], F16, kind="ExternalOutput")

    with tile.TileContext(nc) as tc:
        with (
            tc.tile_pool(name="persist", bufs=1) as persist,
            tc.tile_pool(name="small", bufs=1) as small,
            tc.tile_pool(name="ptp", bufs=1) as ptp,
            tc.tile_pool(name="stp", bufs=2, space="PSUM") as stp,
            tc.tile_pool(name="ctxp", bufs=3, space="PSUM") as ctxp,
            tc.tile_pool(name="prjp", bufs=3, space="PSUM") as prjp,
            tc.tile_pool(name="osb", bufs=4) as osb,
            tc.tile_pool(name="usb", bufs=2) as usb,
            tc.tile_pool(name="lsb", bufs=1) as lsb,
        ):
            xt_sb = persist.tile([P, HT, S], BF16, tag="xt")   # raw X.T, global
            xn_sb = persist.tile([P, KT, H], BF16, tag="xn")   # raw X, natural
            gt_sb = persist.tile([P, HT, QH], BF16, tag="gt")  # G.T = (X@M).T
            wv_sb = persist.tile([P, HT, H], BF16, tag="wv")   # Wv.T
            m_sb = persist.tile([P, HT, H], BF16, tag="m")     # M

            colb = small.tile([P, KT], F32, tag="colb")
            nc.sync.dma_start(colb, col_d[:, :])
            if bv_nonzero:
                bv_sb = small.tile([P, HT], F32, tag="bv_sb")
                nc.sync.dma_start(bv_sb, bv_d[:, :])
            if bq_nonzero:
                d_sb = small.tile([P, HT], BF16, tag="d_sb")
                nc.sync.dma_start(d_sb, dvec_d[:, :])

            # ---- DMA stream, ordered by first consumption ----
            # Few, fat transfers: per-call HWDGE overhead throttled the head
            # when this was ~120 small calls.  m is host-transposed to
            # [ot, P, ht*128+c] so one 1536B-line transfer delivers the full
            # stationary set of G output-block ot; G group (qc=0, ot=0)
            # needs only m[:, 0, :] + xt chunk 0, and subsequent ot groups
            # pipeline on the m[ot] stream.
            nc.sync.dma_start(m_sb[:, 0, :], m_d[0, :, :])
            # chunk 0 in two calls so G(qc0, ot0, half-a) gates on ~590KB
            nc.sync.dma_start(xt_sb[:, :, 0 : QC // 2], xt_d[:, :, 0 : QC // 2])
            nc.sync.dma_start(xt_sb[:, :, QC // 2 : QC], xt_d[:, :, QC // 2 : QC])
            for ot in range(1, HT):
                nc.sync.dma_start(m_sb[:, ot, :], m_d[ot, :, :])
            # own-half X.T chunks 1..3 (feed G qc and scores kt 4qc..4qc+3)
            for xc in range(1, NQC):
                nc.sync.dma_start(xt_sb[:, :, ts(xc, QC)], xt_d[:, :, ts(xc, QC)])
            # other-half X.T in two chunks (scores qc0 kt 16..31)
            nc.sync.dma_start(
                xt_sb[:, :, QH : QH + QH // 2], xt_d[:, :, QH : QH + QH // 2]
            )
            nc.sync.dma_start(xt_sb[:, :, QH + QH // 2 : S], xt_d[:, :, QH + QH // 2 : S])
            # raw X (natural) for P@X, in two column halves so PV groups
            # ht 0..2 only gate on the first half
            nc.sync.dma_start(xn_sb[:, :, 0 : 3 * P], xn_d[:, :, 0 : 3 * P])
            nc.sync.dma_start(xn_sb[:, :, 3 * P : H], xn_d[:, :, 3 * P : H])
            # Wv last (first used by out-proj qc0)
            nc.sync.dma_start(wv_sb[:, :, :], wvt_d[:, :, :])

            # ~6us of dummy matmuls while the first DMAs land: the PE clock
            # is HAM-throttled to 1.2GHz until it has been busy for one
            # ~3.4us activity window, so warm it up on junk data and the
            # real work starts at 2.4GHz.
            warm = small.tile([P, QC], BF16, tag="warm")
            nc.vector.memset(warm, 0.0)
            wps = stp.tile([P, QC], F32, tag="st", name="wps")
            for i in range(WARM_MMS):
                nc.tensor.matmul(
                    wps, warm[:, 0:P], warm, start=(i == 0), stop=(i == WARM_MMS - 1)
                )

            pts = [None] * NQC

            def score_group(qc, kt):
                st_ps = stp.tile([P, QC], F32, tag="st", name="st_ps")
                for ot in range(HT):
                    nc.tensor.matmul(
                        st_ps,
                        xt_sb[:, ot, ts(kt, P)],
                        gt_sb[:, ot, ts(qc, QC)],
                        start=(ot == 0),
                        stop=(ot == HT - 1),
                    )
                nc.scalar.activation(
                    pts[qc][:, kt, :], st_ps, AF.Exp,
                    bias=colb[:, kt : kt + 1], scale=1.0,
                )

            # ---- head: G projection interleaved with scores(qc=0) of the
            # own-half key tiles that ride the same arriving X.T chunks ----
            pts[0] = ptp.tile([P, KT, QC], BF16, tag="pt", name="pt")

            def g_group(qc, ot, lo, w):
                pps = stp.tile([P, QC], F32, tag="st", name="pps")
                for ht in range(HT):
                    nc.tensor.matmul(
                        pps[:, 0:w],
                        m_sb[:, ot, ts(ht, P)],
                        xt_sb[:, ht, qc * QC + lo : qc * QC + lo + w],
                        start=(ht == 0),
                        stop=(ht == HT - 1),
                    )
                nc.vector.tensor_copy(
                    gt_sb[:, ot, qc * QC + lo : qc * QC + lo + w], pps[:, 0:w]
                )

            for qc in range(NQC):
                for ot in range(HT):
                    if qc == 0:
                        # halve the first chunk so the very first group only
                        # gates on ~590KB of the arriving HBM stream
                        g_group(qc, ot, 0, QC // 2)
                        g_group(qc, ot, QC // 2, QC // 2)
                    else:
                        g_group(qc, ot, 0, QC)
                for kt in range(4 * qc, 4 * qc + 4):
                    score_group(0, kt)

            if bq_nonzero:
                # per-key scalar c[k] = X[k] . d folded into the exp bias
                for kt in range(KT):
                    cpps = prjp.tile([P, 1], F32, tag="prj", name="cpps")
                    for ht in range(HT):
                        nc.tensor.matmul(
                            cpps,
                            xt_sb[:, ht, ts(kt, P)],
                            d_sb[:, ht : ht + 1],
                            start=(ht == 0),
                            stop=(ht == HT - 1),
                        )
                    nc.vector.tensor_tensor(
                        colb[:, kt : kt + 1], colb[:, kt : kt + 1], cpps,
                        mybir.AluOpType.add,
                    )

            # ---- attention ----
            for qc in range(NQC):
                if qc > 0:
                    pts[qc] = ptp.tile([P, KT, QC], BF16, tag="pt", name="pt")
                    for kt in range(KT):
                        score_group(qc, kt)
                else:
                    for kt in range(16, KT):
                        score_group(0, kt)
                pt = pts[qc]

                # l[q] = sum_k P.T[k, q]: partial sums on the (idle)
                # vector engine
                lacc = lsb.tile([P, QC], F32, tag="lacc", name="lacc", bufs=2)
                nc.vector.tensor_copy(lacc, pt[:, 0, :])
                for kt in range(1, KT):
                    nc.vector.tensor_tensor(
                        lacc, lacc, pt[:, kt, :], mybir.AluOpType.add
                    )

                # softmax normalizer, entirely off the PE: gpsimd
                # all-reduces lacc across partitions (result in every
                # partition), scalar does 1/l = Exp(-Ln(l)) elementwise.
                lbc = lsb.tile([P, QC], F32, tag="lbc", name="lbc")
                nc.gpsimd.partition_all_reduce(
                    lbc, lacc, 128, bass_isa.ReduceOp.add
                )
                lnl = lsb.tile([P, QC], F32, tag="lnl", name="lnl")
                nc.scalar.activation(lnl, lbc, AF.Ln, scale=1.0)
                bc_sb = lsb.tile([P, QC], F32, tag="bc_sb", name="bc_sb", bufs=2)
                nc.scalar.activation(bc_sb, lnl, AF.Exp, scale=-1.0)

                # U.T[h, q] = X.T-natural @ P.T (P contracted against raw
                # X; Wv applied afterwards to 2048 queries, not 4096 keys)
                # ctx.T[o, q] = Wv @ U.T; normalize + bv on evacuation.
                # The final chunk is processed in two query halves so the
                # last output DMA left dangling after the last matmul is
                # ~65KB, not ~786KB.
                def pv_proj(lo, w):
                    u_sb = usb.tile([P, HT, QC], BF16, tag="u", name="u_sb")
                    for ht in range(HT):
                        ups = ctxp.tile([P, QC], F32, tag="u_ps", name="ups")
                        for kt in range(KT):
                            nc.tensor.matmul(
                                ups[:, 0:w],
                                xn_sb[:, kt, ts(ht, P)],
                                pt[:, kt, lo : lo + w],
                                start=(kt == 0),
                                stop=(kt == KT - 1),
                            )
                        nc.any.tensor_copy(u_sb[:, ht, 0:w], ups[:, 0:w])
                    for ot in range(HT):
                        cps = prjp.tile([P, QC], F32, tag="prj", name="cps")
                        for ht in range(HT):
                            nc.tensor.matmul(
                                cps[:, 0:w],
                                wv_sb[:, ht, ts(ot, P)],
                                u_sb[:, ht, 0:w],
                                start=(ht == 0),
                                stop=(ht == HT - 1),
                            )
                        o_sb = osb.tile([P, QC], F16, tag="o", name="o_sb")
                        nc.vector.tensor_tensor(
                            o_sb[:, 0:w], cps[:, 0:w], bc_sb[:, lo : lo + w],
                            mybir.AluOpType.mult,
                        )
                        if bv_nonzero:
                            nc.vector.tensor_scalar_add(
                                o_sb[:, 0:w], o_sb[:, 0:w], bv_sb[:, ot : ot + 1]
                            )
                        nc.sync.dma_start(
                            out_d[ot, :, qc * QC + lo : qc * QC + lo + w],
                            o_sb[:, 0:w],
                        )

                if qc == NQC - 1:
                    pv_proj(0, QC // 2)
                    pv_proj(QC // 2, QC // 2)
                else:
                    pv_proj(0, QC)
    nc.finalize()
    return nc


# ------------------------- host side -------------------------

_NC_CACHE = {}
TRACE = False
TRACE_CORES = [0]
_LAST_RESULTS = None


def _get_nc(S, H, QH, bv_nonzero, bq_nonzero):
    key = (S, H, QH, bv_nonzero, bq_nonzero)
    if key not in _NC_CACHE:
        _NC_CACHE[key] = build_attention_bass(
            S, H, QH, bv_nonzero=bv_nonzero, bq_nonzero=bq_nonzero
        )
    return _NC_CACHE[key]


def kernel(hidden_states, attention_mask, entity_positions, Wq, bq, Wk, bk, Wv, bv):
    hs = np.asarray(hidden_states, dtype=np.float32)
    am = np.asarray(attention_mask, dtype=np.float32)
    ep = np.asarray(entity_positions)
    Wq = np.asarray(Wq, dtype=np.float32)
    Wk = np.asarray(Wk, dtype=np.float32)
    Wv = np.asarray(Wv, dtype=np.float32)
    bq = np.asarray(bq, dtype=np.float32)
    bv = np.asarray(bv, dtype=np.float32)
    # bk only shifts each query row's scores by a constant -> softmax-invariant

    B, S, H = hs.shape
    QH = S // 2
    HT = H // P
    KT = S // P
    OKT = QH // P
    scale = 1.0 / math.sqrt(H)

    # per-key-column additive term: entity bias (+1 per entity occurrence,
    # duplicates accumulate) + mask
    bias_cols = np.zeros((B, S), dtype=np.float32)
    np.add.at(bias_cols, (np.arange(B)[:, None], ep.astype(np.int64)), 1.0)
    col_add = bias_cols + (1.0 - am) * (-10000.0)

    M = (Wq.T @ Wk) * scale                      # [h, h']
    bv_nonzero = bool(np.any(bv != 0.0))
    bq_nonzero = bool(np.any(bq != 0.0))

    # m transposed to [ot, p, ht*128+c] = M[ht*128+p, ot*128+c]: one fat
    # contiguous transfer per G output block
    m_t = np.ascontiguousarray(
        M.reshape(HT, P, HT, P).transpose(2, 1, 0, 3).reshape(HT, P, H)
    )
    shared = {
        "m": m_t.astype(ml_dtypes.bfloat16),
        "wvt": np.ascontiguousarray(
            Wv.T.reshape(HT, P, H).transpose(1, 0, 2)
        ).astype(ml_dtypes.bfloat16),
    }
    if bq_nonzero:
        dvec = (Wk.T @ bq) * scale               # [h]
        shared["dvec"] = np.ascontiguousarray(
            dvec.reshape(HT, P).T.astype(ml_dtypes.bfloat16)
        )
    if bv_nonzero:
        shared["bv2"] = np.ascontiguousarray(bv.reshape(HT, P).T, dtype=np.float32)

    n_cores = 2 * B
    xt_fulls = [
        np.ascontiguousarray(
            hs[b].T.reshape(HT, P, S).transpose(1, 0, 2)
        ).astype(ml_dtypes.bfloat16)
        for b in range(B)
    ]
    xn_fulls = [
        np.ascontiguousarray(
            hs[b].reshape(KT, P, H).transpose(1, 0, 2)
        ).astype(ml_dtypes.bfloat16)
        for b in range(B)
    ]
    col_ts = [
        np.ascontiguousarray(col_add[b].reshape(KT, P).T, dtype=np.float32)
        for b in range(B)
    ]
    in_maps = []
    for core in range(n_cores):
        b, half = core // 2, core % 2
        if half == 0:
            d = {"xt": xt_fulls[b], "xn": xn_fulls[b], "col": col_ts[b]}
        else:
            # rotate the key axis so this core's queries sit at [0, QH)
            d = {
                "xt": np.ascontiguousarray(
                    np.concatenate(
                        [xt_fulls[b][:, :, QH:], xt_fulls[b][:, :, :QH]], axis=2
                    )
                ),
                "xn": np.ascontiguousarray(
                    np.concatenate([xn_fulls[b][:, OKT:], xn_fulls[b][:, :OKT]], axis=1)
                ),
                "col": np.ascontiguousarray(
                    np.concatenate([col_ts[b][:, OKT:], col_ts[b][:, :OKT]], axis=1)
                ),
            }
        d.update(shared)
        in_maps.append(d)

    nc = _get_nc(S, H, QH, bv_nonzero, bq_nonzero)
    kw = {}
    if TRACE:
        kw = dict(trace=True, trace_cores=list(TRACE_CORES))
    # the accelerator occasionally throws a transient
    # NRT_EXEC_UNIT_UNRECOVERABLE; a clean retry succeeds
    last_exc = None
    for _attempt in range(3):
        try:
            res = run_bass_kernel_spmd(
                nc, in_maps, core_ids=list(range(n_cores)), **kw
            )
            break
        except Exception as e:  # noqa: BLE001
            last_exc = e
    else:
        raise last_exc
    global _LAST_RESULTS
    _LAST_RESULTS = res

    out = np.empty((B, S, H), dtype=np.float32)
    for core in range(n_cores):
        b, half = core // 2, core % 2
        ctx_t = res.results[core]["out"].astype(np.float32).reshape(H, QH)  # [o, q]
        out[b, half * QH : (half + 1) * QH, :] = ctx_t.T
    return out
